# revision 79
# baseline (speedup 1.0000x reference)
"""Trainium2 Bass kernel for nn_CADense (context-adaptive low-rank dense layer).

Computes, for the full batch:
    s_mod = s + context @ w          # [B, R]
    low   = (data @ u) * s_mod       # [B, R]
    out   = relu(low @ v.T + 2*bias) # [B, UNITS]

Sharding: data-parallel over batch across 8 NeuronCores; u/s/v/w/bias
replicated. Each core runs the same Bass program on its 1024-row shard.

All wire traffic is bf16 (inputs cast host-side; output stored bf16 and
upcast host-side), which halves HBM bytes vs f32 — the kernel sits at
the HBM/PE ridge, so this is the dominant win. PSUM accumulation stays
f32 and the s_mod add/multiply run in f32; max relative error ~6e-3.

The whole computation runs in the transposed ("T") domain:
    pdT[r, b]   = (u.T @ data.T)[r, b]            (PE, 16 k-chunks)
    smodT[r, b] = s[r] + (w.T @ ctx.T)[r, b]      (PE + scalar-engine add)
    lowT[r, b]  = pdT * smodT                     (vector engine, bf16 out)
    outT[m, b]  = relu((v.T).T @ lowT + 2*bias[m])
Keeping the output transposed makes 2*bias a PER-PARTITION scalar, so it
folds into the ReLU evacuation (scalar-engine activation bias / DVE
tensor_scalar) instead of costing 16 extra K=1 matmuls. The host
un-transposes the [UNITS, NB] result when unsharding.

Schedule notes:
- The kernel is PE-serial-bound (144 real matmuls x 512 rows ~ 31 us at
  2.4 GHz), so the schedule exists to (a) start the PE as early as the
  preamble allows, (b) never starve it, (c) drain the tail fast.
- Loads split across both HWDGE rings by WHEN the PE needs them: the sync
  ring streams the eight 512 KiB dataT chunks (plus u's back half slotted
  mid-stream and vT at the end); the scalar ring carries only 780 KiB of
  early weights (u front/w/s/bias) and then ctx. Data chunks must NOT ride
  the scalar ring — that wedges the device. A ring entry carries ~0.9 us
  of unoverlapped spin-up, so entry count/granularity is load-balanced
  against completion laziness (8x512 KiB beat both 4x1 MiB and 16x256 KiB).
- PSUM is one pool of four 2-bank [128, 2, 512] tiles shared by warm-up,
  smod, the two pd accumulators and the 16 rotating output tiles: the out
  stage gets depth-4 pipelining, and each output unit-chunk evacuates with
  a SINGLE wide ReLU+bias instruction ([128, 1024], per-partition bias),
  alternating scalar/vector engines.
- The final rank group runs bt-major with the lowT muls emitted inline so
  the DVE multiplies batch-half 0 while the PE finishes batch-half 1.
- Stores: one 256 KiB store per unit chunk, all via the sync ring (its
  engine is idle once loads are triggered; the out stage is PE-paced so a
  single store ring keeps up, and involving the gpsimd SWDGE ring measured
  ~0.7 us slower end-to-end); the last two chunks evacuate and store in
  halves so the final chain is half-latency.
- Weights are host-packed partition-major (u/w/vT/s+bias) so every DMA
  moves contiguous 2-8 KiB per-partition lines.
"""

import os
import sys
from contextlib import ExitStack

import numpy as np


def _ensure_concourse():
    try:
        import concourse  # noqa: F401
    except ImportError:
        for p in ("/opt/trn_rl_repo", "/root/.axon_site/_ro/trn_rl_repo"):
            if os.path.isdir(p) and p not in sys.path:
                sys.path.insert(0, p)


_ensure_concourse()

import concourse.tile as tile  # noqa: E402
from concourse import bacc, mybir  # noqa: E402
from concourse.bass_utils import run_bass_kernel_spmd  # noqa: E402

NCORES = 8
B, N_IN, UNITS, RANK, CCTX = 8192, 2048, 2048, 256, 512
NB = B // NCORES  # batch rows per core
P = 128
BT = 512  # batch tile (PSUM bank / moving-dim limit)
NBT = NB // BT  # 2 batch tiles per core
KC = N_IN // P  # 16 contraction chunks for data @ u
CC = CCTX // P  # 4 contraction chunks for context @ w
RC = RANK // P  # 2 rank chunks
UC = UNITS // P  # 16 output unit chunks
N_WARMUP_MM = 8

F32 = mybir.dt.float32
BF16 = mybir.dt.bfloat16
RELU = mybir.ActivationFunctionType.Relu
ALU_ADD = mybir.AluOpType.add
ALU_MAX = mybir.AluOpType.max


def _emit(nc, tc, ctx):
    # Host-marshaled layouts (built per-shard in kernel()):
    #   dataT = data.T; ctxT = context.T
    #   u_p[p, kc*RANK + r]   = u[kc*128 + p, r]
    #   w_p[p, cc*RANK + r]   = w[cc*128 + p, r]
    #   vT_p[p, rc*UNITS + m] = v[m, rc*128 + p]
    #   sb_p[p, 0:2] = s[rc*128 + p]; sb_p[p, 2+uc] = bias[uc*128 + p]
    d_dataT = nc.dram_tensor("dataT", [N_IN, NB], BF16, kind="ExternalInput")
    d_ctxT = nc.dram_tensor("ctxT", [CCTX, NB], BF16, kind="ExternalInput")
    d_u = nc.dram_tensor("u_p", [P, KC * RANK], BF16, kind="ExternalInput")
    d_w = nc.dram_tensor("w_p", [P, CC * RANK], BF16, kind="ExternalInput")
    d_vT = nc.dram_tensor("vT_p", [P, RC * UNITS], BF16, kind="ExternalInput")
    d_sb = nc.dram_tensor("sb_p", [P, RC + UC], F32, kind="ExternalInput")
    d_outT = nc.dram_tensor("outT", [UNITS, NB], BF16, kind="ExternalOutput")

    ap_dataT = d_dataT.ap().rearrange("(g p) b -> p g b", p=P)  # [128, 16, NB]
    ap_ctxT = d_ctxT.ap().rearrange("(cc p) b -> p cc b", p=P)  # [128, 4, NB]
    ap_u = d_u.ap().rearrange("p (kc r) -> p kc r", r=RANK)
    ap_w = d_w.ap().rearrange("p (cc r) -> p cc r", r=RANK)
    ap_vT = d_vT.ap().rearrange("p (rc m) -> p rc m", m=UNITS)
    ap_outT = d_outT.ap().rearrange("(uc p) b -> p uc b", p=P)

    singles = ctx.enter_context(tc.tile_pool(name="singles", bufs=1))
    dpool = ctx.enter_context(tc.tile_pool(name="dataT", bufs=1))
    # ALL of PSUM is one pool of four 2-bank [128, 2, 512] tiles sharing one
    # tag: warm-up, the two pd accumulators, the two smod tiles and the 16
    # output tiles rotate through the same four buffers, so the out stage
    # gets depth-4 pipelining once the earlier tiles die.
    o_psum = ctx.enter_context(tc.tile_pool(name="o_psum", bufs=4, space="PSUM"))
    lowpool = ctx.enter_context(tc.tile_pool(name="lowT", bufs=1))
    smodpool = ctx.enter_context(tc.tile_pool(name="smod", bufs=1))
    opool = ctx.enter_context(tc.tile_pool(name="outsb", bufs=6))

    # HAM warm-up fodder: garbage bf16 matmuls while the first loads stream.
    wu_a = singles.tile([P, P], BF16)
    nc.vector.memset(wu_a[:], 1.0)
    wu_b = singles.tile([P, BT], BF16)
    nc.vector.memset(wu_b[:], 1.0)

    # ---- input DMAs ----------------------------------------------------
    # sync ring: dataT chunks only, 256 KiB apiece (first-use order; small
    # transfers so each completion semaphore fires early and the PE-paced
    # rank stage starts as soon as possible).
    # Data chunks alternate across BOTH HWDGE rings: each ring entry carries
    # ~0.9 us of unoverlapped spin-up, so a single ring delivers 512 KiB
    # chunks ~2.3 us apart while the PE consumes them in ~1.9 us. Two rings
    # in flight halve the effective spacing. ctx/vT queue BEHIND the data
    # (not needed until the contraction ends); u/w/s ride the scalar ring
    # interleaved so the next lhsT always lands before its data.
    dqoff = [2 * g for g in range(8)]
    dqsizes = [2] * 8
    dq = [
        dpool.tile([P, n, NB], BF16, tag=f"dq{g}", name=f"dq{g}")
        for g, n in enumerate(dqsizes)
    ]
    kc2g = {}
    for g, (o, n) in enumerate(zip(dqoff, dqsizes)):
        for j in range(n):
            kc2g[o + j] = (g, j)
    u_sb = singles.tile([P, KC, RANK], BF16)
    w_sb = singles.tile([P, CC, RANK], BF16)
    sb_sb = singles.tile([P, RC + UC], F32)
    ctx_sb = singles.tile([P, CC, NB], BF16)
    vT_sb = singles.tile([P, RC, UNITS], BF16)

    # sync ring: the data stream, with u's back half slotted mid-stream
    # (needed only from k-chunk 8). scalar ring: just 768 KiB of early
    # weights, then ctx — so data and weights never fight for HBM in the
    # window where the PE is consuming data chunks at full rate.
    for g in range(4):
        nc.sync.dma_start(
            out=dq[g][:], in_=ap_dataT[:, dqoff[g] : dqoff[g] + dqsizes[g]]
        )
    nc.sync.dma_start(out=u_sb[:, 8:], in_=ap_u[:, 8:])
    for g in range(4, len(dqsizes)):
        nc.sync.dma_start(
            out=dq[g][:], in_=ap_dataT[:, dqoff[g] : dqoff[g] + dqsizes[g]]
        )
    nc.sync.dma_start(out=vT_sb[:, 0], in_=ap_vT[:, 0])
    nc.sync.dma_start(out=vT_sb[:, 1], in_=ap_vT[:, 1])
    nc.scalar.dma_start(out=u_sb[:, 0:2], in_=ap_u[:, 0:2])
    nc.scalar.dma_start(out=u_sb[:, 2:8], in_=ap_u[:, 2:8])
    nc.scalar.dma_start(out=w_sb[:], in_=ap_w)
    nc.scalar.dma_start(out=sb_sb[:], in_=d_sb.ap())
    nc.scalar.dma_start(out=ctx_sb[:], in_=ap_ctxT)

    # 2*bias, computed once on the vector engine (idle during load phase).
    bias2 = singles.tile([P, UC], F32)
    nc.vector.tensor_scalar_mul(bias2[:], sb_sb[:, RC:], 2.0)

    # ---- HAM warm-up ---------------------------------------------------
    wu_ps = o_psum.tile([P, NBT, BT], F32, tag="po", name="wu_ps")
    for _ in range(N_WARMUP_MM):
        nc.tensor.matmul(
            wu_ps[:, 0], lhsT=wu_a[:], rhs=wu_b[:], start=True, stop=True
        )

    # ---- rank stage: pdT[r, b] = sum_kc u[kc].T @ dataT[kc] ------------
    # pd[bt] is a 2-bank tile; the rc halves are separate accumulation
    # regions.
    pd = {
        bt: o_psum.tile([P, RC, BT], F32, tag="po", name=f"pd{bt}")
        for bt in range(NBT)
    }

    def rank_mm(kc, bt, rc):
        nc.tensor.matmul(
            pd[bt][:, rc],
            lhsT=u_sb[:, kc, rc * P : (rc + 1) * P],
            rhs=dq[kc2g[kc][0]][:, kc2g[kc][1], bt * BT : (bt + 1) * BT],
            start=(kc == 0),
            stop=(kc == KC - 1),
        )

    def rank_group(g):
        for kc4 in range(4):
            for bt in range(NBT):
                for rc in range(RC):
                    rank_mm(g * 4 + kc4, bt, rc)

    rank_group(0)
    rank_group(1)

    # smod: ctx @ w matmuls + s-add. Emitted after rank group 1 — the exact
    # window where the data stream has fallen behind the PE (ring entries
    # arrive ~2.3 us apart vs ~1.7 us consumption): these 16 matmuls absorb
    # the deficit so groups 2-3 then run fully fed. Earlier placement stalls
    # on ctx and resurfaces the deficit at dq7; later placement stalls on
    # ctx after the data is done. Both measured worse.
    smod = {}
    for bt in range(NBT):
        ps = o_psum.tile([P, RC, BT], F32, tag="po", name=f"ps{bt}")
        for rc in range(RC):
            for cc in range(CC):
                nc.tensor.matmul(
                    ps[:, rc],
                    lhsT=w_sb[:, cc, rc * P : (rc + 1) * P],
                    rhs=ctx_sb[:, cc, bt * BT : (bt + 1) * BT],
                    start=(cc == 0),
                    stop=(cc == CC - 1),
                )
        for rc in range(RC):
            sm = smodpool.tile([P, BT], F32, tag=f"smod{bt}{rc}", name="smod")
            nc.scalar.add(sm[:], ps[:, rc], add=sb_sb[:, rc : rc + 1])
            smod[(bt, rc)] = sm

    rank_group(2)

    # Final rank group runs bt-major so bt0's accumulators close first and
    # their lowT muls (DVE is the only elementwise engine with PSUM access)
    # overlap the PE finishing bt1's contraction.
    lowT = {
        bt: lowpool.tile([P, RC, BT], BF16, tag=f"low{bt}", name=f"low{bt}")
        for bt in range(NBT)
    }
    for bt in range(NBT):
        for kc in range(12, 16):
            for rc in range(RC):
                rank_mm(kc, bt, rc)
        for rc in range(RC):
            nc.vector.tensor_mul(
                out=lowT[bt][:, rc], in0=pd[bt][:, rc], in1=smod[(bt, rc)]
            )

    # ---- output stage: outT[uc] = relu(vT[uc].T @ lowT + 2*bias) -------
    # po[uc] spans both batch halves (2 banks); ONE wide ReLU+bias
    # evacuation per unit chunk, alternating scalar/vector engines; one
    # 256 KiB store per unit chunk on the sync ring (idle after loads).
    for uc in range(UC):
        osb = opool.tile([P, NB], BF16, tag="osb", name="osb")
        po = o_psum.tile([P, NBT, BT], F32, tag="po", name="po")
        for bt in range(NBT):
            for rc in range(RC):
                nc.tensor.matmul(
                    po[:, bt],
                    lhsT=vT_sb[:, rc, uc * P : (uc + 1) * P],
                    rhs=lowT[bt][:, rc],
                    start=(rc == 0),
                    stop=(rc == RC - 1),
                )
        if uc >= UC - 2:
            # Tail drain: split the final two evacuations in half across
            # both engines (interleaved so neither engine holds the kernel's
            # last item long) and store each half immediately via the idle
            # sync ring — the kernel's last evac+store chain is half-latency.
            e0 = nc.vector if uc == UC - 2 else nc.scalar
            if e0 is nc.scalar:
                nc.scalar.activation(
                    osb[:, 0:BT], po[:, 0], RELU, bias=bias2[:, uc : uc + 1]
                )
                nc.vector.tensor_scalar(
                    osb[:, BT:], po[:, 1], bias2[:, uc : uc + 1], 0.0,
                    ALU_ADD, ALU_MAX,
                )
            else:
                nc.vector.tensor_scalar(
                    osb[:, 0:BT], po[:, 0], bias2[:, uc : uc + 1], 0.0,
                    ALU_ADD, ALU_MAX,
                )
                nc.scalar.activation(
                    osb[:, BT:], po[:, 1], RELU, bias=bias2[:, uc : uc + 1]
                )
            nc.sync.dma_start(out=ap_outT[:, uc, 0:BT], in_=osb[:, 0:BT])
            nc.sync.dma_start(out=ap_outT[:, uc, BT:], in_=osb[:, BT:])
            continue
        # uc 12-13 go to the scalar engine so the vector engine's queue is
        # short when the tail-drain halves arrive.
        if uc % 2 == 0 or uc >= UC - 4:
            nc.scalar.activation(
                osb[:], po[:].rearrange("p a b -> p (a b)"), RELU,
                bias=bias2[:, uc : uc + 1],
            )
        else:
            nc.vector.tensor_scalar(
                osb[:], po[:].rearrange("p a b -> p (a b)"),
                bias2[:, uc : uc + 1], 0.0, ALU_ADD, ALU_MAX,
            )
        nc.sync.dma_start(out=ap_outT[:, uc], in_=osb[:])


_CACHE = {}


def build():
    if "nc" in _CACHE:
        return _CACHE["nc"]
    nc = bacc.Bacc("TRN2", target_bir_lowering=False, debug=False)
    with tile.TileContext(nc) as tc, ExitStack() as ctx:
        _emit(nc, tc, ctx)
    nc.compile()
    _CACHE["nc"] = nc
    return nc


def make_in_maps(data, context, u, s, v, w, bias):
    import ml_dtypes

    bf16 = ml_dtypes.bfloat16
    u_p = np.ascontiguousarray(
        np.asarray(u, dtype=np.float32)
        .reshape(KC, P, RANK)
        .transpose(1, 0, 2)
        .reshape(P, KC * RANK)
    ).astype(bf16)
    w_p = np.ascontiguousarray(
        np.asarray(w, dtype=np.float32)
        .reshape(CC, P, RANK)
        .transpose(1, 0, 2)
        .reshape(P, CC * RANK)
    ).astype(bf16)
    # vT_p[p, rc*UNITS + m] = v[m, rc*128 + p]
    vT_p = np.ascontiguousarray(
        np.asarray(v, dtype=np.float32)
        .T.reshape(RC, P, UNITS)
        .transpose(1, 0, 2)
        .reshape(P, RC * UNITS)
    ).astype(bf16)
    sb_p = np.concatenate(
        [
            np.asarray(s, dtype=np.float32).reshape(RC, P).T,
            np.asarray(bias, dtype=np.float32).reshape(UC, P).T,
        ],
        axis=1,
    )
    sb_p = np.ascontiguousarray(sb_p)
    data = np.asarray(data, dtype=np.float32)
    context = np.asarray(context, dtype=np.float32)
    in_maps = []
    for c in range(NCORES):
        sl = slice(c * NB, (c + 1) * NB)
        in_maps.append(
            {
                "dataT": np.ascontiguousarray(data[sl].T).astype(bf16),
                "ctxT": np.ascontiguousarray(context[sl].T).astype(bf16),
                "u_p": u_p,
                "w_p": w_p,
                "vT_p": vT_p,
                "sb_p": sb_p,
            }
        )
    return in_maps


def kernel(data, context, u, s, v, w, bias):
    nc = build()
    in_maps = make_in_maps(data, context, u, s, v, w, bias)
    res = run_bass_kernel_spmd(nc, in_maps, core_ids=list(range(NCORES)))
    out = np.empty((B, UNITS), dtype=np.float32)
    for c in range(NCORES):
        out[c * NB : (c + 1) * NB] = res.results[c]["outT"].T.astype(np.float32)
    return out
